# revision 3
# baseline (speedup 1.0000x reference)
"""Trainium2 Bass kernel for complex-valued channel attention (XCA-style) — v4.

Two-pass restructure of v3, designed around the fact that walrus compiles
with --enable-ldw-opt=false: EVERY matmul reloads its stationary weights,
so the kernel keeps each weight load shorter than the compute issued per
load (2+ matmuls per load), instead of the baseline's per-row qk matmuls
(one 256-col DR load per 129-cycle matmul).

  Pass A (per batch): q,k conv in fp8 DoubleRow over FLAT 512-wide chunks
    of the padded plane (output positions at row-crossings are garbage and
    simply never read; token blocks of 128 stay contiguous).  Pair-major
    over 2-chunk groups: one DR weight load feeds two 256-cycle matmuls.
    Transposes + QK^T/QQ^T accumulation pipeline behind the conv exactly
    as before ([kT|qT]-interleaved fp8 blocks, DR token-pair matmuls).

  Fold: S = attn @ w_proj (softmax stages on DVE/ACT, fast-rsqrt norms on
    DVE — no ACT table switches), then W2[i,tap,o] = sum_c Wv[i,tap,c]
    S[c,o] built on-device with 9 small matmuls.

  Pass B (per batch): y = W2-conv(x) directly — the 9-tap bf16 v-conv with
    attn+proj pre-folded into its weights.  Tap-major over 2-tile groups
    (one 128-col load per two 512-row matmuls), output evicted straight to
    the y staging buffer.  The separate attn@v tail phase, the V SBUF
    buffer, and its PSUM evictions are gone.
"""

import sys

sys.path.insert(0, '/opt/trn_rl_repo')

import numpy as np

import concourse.bass as bass  # noqa: F401  (registers bass types)
import concourse.tile as tile
from concourse import bacc, mybir
from concourse.ap import AP
from concourse.bass_utils import run_bass_kernel_spmd
from concourse.masks import make_identity

F32 = mybir.dt.float32
BF16 = mybir.dt.bfloat16
FP8 = mybir.dt.float8e4
I32 = mybir.dt.int32

B, C, W, H = 2, 64, 256, 256
NCORES = 8
WL = W // NCORES          # 32 local w rows per core
HP = H + 2                # 258: h with zero pad columns
WLH = WL + 2              # 34: local w rows + halo
NWT = WL // 2             # 16 tiles of 512 tokens (2 w-rows x 256 h)
FLAT = WL * HP            # 8256 flat conv output positions per batch
EPS = 1e-12
WSCALE = 64.0             # q,k fp8 weights are scaled by 2^6

# tap order: it = 3*k0 + k1;  DR pairs (0,1) (2,3) (4,5) (6,7) (8,zero)
TAPS = [(k0, k1) for k0 in range(3) for k1 in range(3)]

# pass-A chunk groups: 8 pairs of 512-flat chunks (one DR weight load per
# two matmuls), then the 64-wide tail chunk solo (reuses the pair tags)
GROUPS = [((2 * g, 512), (2 * g + 1, 512)) for g in range(8)]
GROUPS.append(((16, 64),))

_CACHE = {}


def _build(reps=0, hwloop=True):
    """Emit + compile the 8-core SPMD program. reps>0 wraps the compute in a
    hardware loop (used only for timing; collectives become local copies)."""
    nc = bacc.Bacc(None, target_bir_lowering=False, debug=False,
                   num_devices=NCORES)
    x_bf = nc.declare_dram_parameter("x_bf", [B, 128, WLH, HP], BF16,
                                     isOutput=False)
    x_f8 = nc.declare_dram_parameter("x_f8", [B, 128, WLH, HP], FP8,
                                     isOutput=False)
    w_qk = nc.declare_dram_parameter("w_qk", [128, 2, 10, 128], FP8,
                                     isOutput=False)
    w_vt = nc.declare_dram_parameter("w_vt", [128, 9, 128], BF16,
                                     isOutput=False)
    w_proj = nc.declare_dram_parameter("w_proj", [128, 128], BF16,
                                       isOutput=False)
    y_out = nc.declare_dram_parameter("y_out", [B, 128, WL, H], BF16,
                                      isOutput=True)

    with tile.TileContext(nc) as tc:
        with (
            tc.tile_pool(name="const", bufs=1) as const,
            tc.tile_pool(name="xp", bufs=1) as xp,
            tc.tile_pool(name="qs", bufs=1) as qsp,
            tc.tile_pool(name="qkt", bufs=3) as qkt,
            tc.tile_pool(name="w2p", bufs=1) as w2p,
            tc.tile_pool(name="scr", bufs=2) as scr,
            tc.tile_pool(name="stat", bufs=1) as stat,
            tc.tile_pool(name="dram", bufs=1, space="DRAM") as dram,
            tc.tile_pool(name="psacc", bufs=1, space="PSUM") as psacc,
        ):
            wqk = const.tile([128, 2, 10, 128], FP8)
            nc.sync.dma_start(out=wqk[:], in_=w_qk[:])
            wvt = const.tile([128, 9, 128], BF16)
            nc.sync.dma_start(out=wvt[:], in_=w_vt[:])
            wp = const.tile([128, 128], BF16)
            nc.sync.dma_start(out=wp[:], in_=w_proj[:])
            identf = const.tile([128, 128], F32)
            make_identity(nc, identf[:])
            identb = const.tile([128, 128], BF16)
            nc.vector.tensor_copy(out=identb[:], in_=identf[:])

            # pass A reads the flat fp8 copy (with a 2-element guard for
            # pad-position reads past the data end); pass B the bf16 one.
            # fp8 chunks stream first: they gate time-to-first-matmul.
            X = [xp.tile([128, WLH, HP], BF16, tag=f"x{b}", name=f"X{b}")
                 for b in range(B)]
            X8 = [xp.tile([128, WLH * HP + 2], FP8, tag=f"x8{b}",
                          name=f"X8{b}")
                  for b in range(B)]
            CHUNKS = ((0, 4), (4, 8), (8, 14), (14, 21), (21, 28),
                      (28, WLH))
            for b in range(B):
                for lo, hi in CHUNKS:
                    x8v = X8[b][:, lo * HP:hi * HP].rearrange(
                        "p (w h) -> p w h", h=HP)
                    nc.sync.dma_start(out=x8v, in_=x_f8[b, :, lo:hi, :])
                nc.vector.memset(X8[b][:, WLH * HP:WLH * HP + 2], 0)
            for b in range(B):
                for lo, hi in CHUNKS:
                    nc.sync.dma_start(out=X[b][:, lo:hi, :],
                                      in_=x_bf[b, :, lo:hi, :])

            # flat fp8 conv outputs (token rows 258 apart, 2 garbage cols
            # per row); one buffer per stream, reused across batches
            q_s8 = qsp.tile([128, FLAT], BF16, name="q_s8")
            k_s8 = qsp.tile([128, FLAT], BF16, name="k_s8")
            # per-batch QK accumulator banks (double as softmax scratch)
            Ab = [psacc.tile([128, 384], F32, tag=f"acc{b}", name=f"A{b}")
                  for b in range(B)]
            S = [stat.tile([128, 128], BF16, tag=f"S{b}", name=f"S{b}")
                 for b in range(B)]
            W2 = [stat.tile([128, 9, 128], BF16, tag=f"W2{b}",
                            name=f"W2{b}") for b in range(B)]
            stats_s = [stat.tile([128, 130], F32, tag=f"st{b}", name=f"st{b}")
                       for b in range(B)]
            rstats = [stat.tile([128, 130], F32, tag=f"rst{b}", name=f"rst{b}")
                      for b in range(B)]
            cc_in = [dram.tile([128, 130], F32, tag=f"ci{b}", name=f"ci{b}")
                     for b in range(B)]
            cc_out = [dram.tile([128, 130], F32, tag=f"co{b}", name=f"co{b}")
                      for b in range(B)]

            def dr_rhs(xt, flat_off, width, pair):
                """[128, 2(tap delta), width] overlapping AP over the flat
                padded plane: DR pair input windows for output flat
                positions [flat_off, flat_off+width)."""
                k0a, k1a = TAPS[pair[0]]
                full = xt[:]
                pstride = full.ap[0][0]
                off = full.offset + flat_off + k0a * HP + k1a
                if pair[1] < 9:
                    k0b, k1b = TAPS[pair[1]]
                    d = (k0b - k0a) * HP + (k1b - k1a)
                else:
                    d = 0
                return AP(full.tensor, off, [[pstride, 128], [d, 2],
                                             [1, width]])

            loop_cm = (tc.For_i(0, reps, 1,
                                hint_engines=(mybir.EngineType.PE,
                                              mybir.EngineType.Activation,
                                              mybir.EngineType.DVE))
                       if reps and hwloop else None)
            if loop_cm is not None:
                loop_cm.__enter__()

            # ---------------- pass A: q,k conv + QK^T ----------------
            pA_cm = tc.tile_pool(name="pA", bufs=1, space="PSUM")
            pA = pA_cm.__enter__()

            qkt_hist = {}

            def conv_group(b, g):
                """fp8 DR conv for one chunk group, pair-major: one weight
                load per 2-3 chunk matmuls."""
                chunks = GROUPS[g]
                pq = [pA.tile([128, 512], F32, tag=f"pq{i}", name=f"pq{i}",
                              bufs=1) for i, _ in enumerate(chunks)]
                pk = [pA.tile([128, 512], F32, tag=f"pk{i}", name=f"pk{i}",
                              bufs=1) for i, _ in enumerate(chunks)]
                for chunk, pt in ((0, pq), (1, pk)):
                    for p in range(5):
                        for i, (c, width) in enumerate(chunks):
                            nc.tensor.matmul(
                                pt[i][:, 0:width],
                                wqk[:, chunk, 2 * p:2 * p + 2, :],
                                dr_rhs(X8[b], 512 * c, width,
                                       (2 * p, 2 * p + 1)),
                                start=(p == 0), stop=(p == 4),
                                perf_mode=mybir.MatmulPerfMode.DoubleRow)
                for i, (c, width) in enumerate(chunks):
                    nc.vector.tensor_scalar_mul(
                        out=q_s8[:, 512 * c:512 * c + width],
                        in0=pq[i][:, 0:width], scalar1=1.0 / WSCALE)
                    nc.scalar.activation(
                        out=k_s8[:, 512 * c:512 * c + width],
                        in_=pk[i][:, 0:width],
                        func=mybir.ActivationFunctionType.Copy,
                        scale=1.0 / WSCALE)

            def tr_stage(b, wt):
                """8 transposes of tile wt's token blocks into one
                [kT|qT]-interleaved fp8 PSUM bank, then evict."""
                ptqk = pA.tile([128, 1024], BF16, tag="ptqk", name="ptqk",
                               bufs=2)
                base = 516 * wt
                offs = (base, base + 128, base + 258, base + 386)
                for j, o in enumerate(offs):
                    nc.tensor.transpose(ptqk[:, 256 * j:256 * j + 128],
                                        k_s8[:, o:o + 128], identb[:])
                    nc.tensor.transpose(ptqk[:, 256 * j + 128:256 * (j + 1)],
                                        q_s8[:, o:o + 128], identb[:])
                QKT = qkt.tile([128, 1024], FP8, tag="QKT", name="QKT")
                nc.scalar.copy(out=QKT[:, 0:512], in_=ptqk[:, 0:512])
                nc.vector.tensor_copy(out=QKT[:, 512:1024],
                                      in_=ptqk[:, 512:1024])
                qkt_hist[wt] = QKT

            def qk_stage(b, wt):
                QKT = qkt_hist.pop(wt)
                blk = QKT[:].rearrange("p (j c) -> p j c", j=4)
                # [QK|QQ] plus KK (its diagonal gives the k norms, so no
                # ACT Square pass loads the eviction engines).  All writes
                # are ONE per-bank accumulation group: start only on the
                # very first matmul, stop only on the very last.
                for j in (0, 2):
                    nc.tensor.matmul(
                        Ab[b][:, 0:256],
                        blk[:, j:j + 2, 128:256], blk[:, j:j + 2, :],
                        start=(wt == 0 and j == 0), stop=False,
                        perf_mode=mybir.MatmulPerfMode.DoubleRow)
                    nc.tensor.matmul(
                        Ab[b][:, 256:384],
                        blk[:, j:j + 2, 0:128], blk[:, j:j + 2, 0:128],
                        start=False,
                        stop=(wt == NWT - 1 and j == 2),
                        perf_mode=mybir.MatmulPerfMode.DoubleRow)

            def pass_a(b, hooks=None):
                # tiles become transposable once the chunks covering their
                # 516-wide flat span are evicted (2-ish per group)
                done_tr = 0
                done_qk = 0
                for g in range(len(GROUPS) + 1):
                    if g < len(GROUPS):
                        conv_group(b, g)
                    hi_flat = 1024 * (g + 1) if g < len(GROUPS) else 99999
                    while done_tr < NWT and 516 * (done_tr + 1) <= hi_flat:
                        tr_stage(b, done_tr)
                        done_tr += 1
                        # qk matmuls lag one tile behind their transpose
                        if done_qk < done_tr - 1:
                            qk_stage(b, done_qk)
                            done_qk += 1
                    if hooks and g in hooks:
                        hooks[g]()
                while done_qk < NWT:
                    qk_stage(b, done_qk)
                    done_qk += 1

            def stats_and_cc(b):
                # local [QK | diag(QQ) | sum k^2] -> AllReduce across cores
                nc.scalar.copy(out=stats_s[b][:, 0:128], in_=Ab[b][:, 0:128])
                dscr = scr.tile([128, 128], F32, tag="dscr", name="dscr")
                nc.vector.tensor_tensor(out=dscr[:], in0=Ab[b][:, 128:256],
                                        in1=identf[:],
                                        op=mybir.AluOpType.mult)
                nc.vector.reduce_sum(out=stats_s[b][:, 128:129], in_=dscr[:],
                                     axis=mybir.AxisListType.X)
                dscr2 = scr.tile([128, 128], F32, tag="dscr2", name="dscr2")
                nc.vector.tensor_tensor(out=dscr2[:], in0=Ab[b][:, 256:384],
                                        in1=identf[:],
                                        op=mybir.AluOpType.mult)
                nc.vector.reduce_sum(out=stats_s[b][:, 129:130],
                                     in_=dscr2[:],
                                     axis=mybir.AxisListType.X)
                if reps:
                    nc.vector.tensor_copy(out=rstats[b][:], in_=stats_s[b][:])
                else:
                    nc.sync.dma_start(out=cc_in[b][:], in_=stats_s[b][:])
                    nc.gpsimd.collective_compute(
                        "AllReduce", mybir.AluOpType.add,
                        replica_groups=[list(range(NCORES))],
                        ins=[cc_in[b].opt()], outs=[cc_out[b].opt()],
                    )
                    nc.sync.dma_start(out=rstats[b][:], in_=cc_out[b][:])

            # ---- softmax stages (split fine so PE ops issue one hook
            # after the V-engine ops they depend on) ----
            def sm_a(b, st):
                # V-engine only: reciprocal norms
                nq = scr.tile([128, 2], F32, tag="nq", name="nq")
                nc.scalar.activation(
                    out=nq[:], in_=rstats[b][:, 128:130],
                    func=mybir.ActivationFunctionType.Sqrt)
                nqm = scr.tile([128, 2], F32, tag="nqm", name="nqm")
                nc.vector.tensor_scalar_max(out=nqm[:], in0=nq[:],
                                            scalar1=EPS)
                rqk = scr.tile([128, 2], F32, tag="rqk", name="rqk",
                               bufs=2)
                nc.vector.reciprocal(out=rqk[:], in_=nqm[:])
                st['rqk'] = rqk

            def sm_b(b, st):
                nc.tensor.transpose(Ab[b][:, 0:128], rstats[b][:, 0:128],
                                    identf[:])
                s1 = scr.tile([128, 128], F32, tag="s1", name="s1", bufs=2)
                nc.scalar.copy(out=s1[:], in_=Ab[b][:, 0:128])
                st['s1'] = s1

            def sm_c(b, st):
                s1b = scr.tile([128, 128], F32, tag="s1b", name="s1b",
                               bufs=2)
                nc.vector.tensor_scalar_mul(out=s1b[:], in0=st['s1'][:],
                                            scalar1=st['rqk'][:, 1:2])
                st['s1b'] = s1b

            def sm_d(b, st):
                nc.tensor.transpose(Ab[b][:, 128:256], st['s1b'][:],
                                    identf[:])
                lg = scr.tile([128, 128], F32, tag="lg", name="lg", bufs=2)
                nc.scalar.copy(out=lg[:], in_=Ab[b][:, 128:256])
                st['lg'] = lg

            def sm_e(b, st):
                lg2 = scr.tile([128, 128], F32, tag="lg2", name="lg2")
                nc.vector.tensor_scalar_mul(out=lg2[:], in0=st['lg'][:],
                                            scalar1=st['rqk'][:, 0:1])
                nc.vector.memset(lg2[0:64, 64:128], -1e30)
                nc.vector.memset(lg2[64:128, 0:64], -1e30)
                mx = scr.tile([128, 1], F32, tag="mx", name="mx")
                nc.vector.reduce_max(out=mx[:], in_=lg2[:],
                                     axis=mybir.AxisListType.X)
                sh = scr.tile([128, 128], F32, tag="sh", name="sh")
                nc.vector.tensor_scalar(out=sh[:], in0=lg2[:], scalar1=mx[:],
                                        scalar2=None,
                                        op0=mybir.AluOpType.subtract)
                ex = scr.tile([128, 128], F32, tag="ex", name="ex")
                esum = scr.tile([128, 1], F32, tag="esum", name="esum")
                nc.scalar.activation(out=ex[:], in_=sh[:],
                                     func=mybir.ActivationFunctionType.Exp,
                                     accum_out=esum[:])
                rs = scr.tile([128, 1], F32, tag="rs", name="rs")
                nc.vector.reciprocal(out=rs[:], in_=esum[:])
                pr = scr.tile([128, 128], BF16, tag="pr", name="pr",
                              bufs=2)
                nc.vector.tensor_scalar_mul(out=pr[:], in0=ex[:],
                                            scalar1=rs[:])
                st['pr'] = pr

            def sm_f(b, st):
                nc.tensor.matmul(Ab[b][:, 128:256], st['pr'][:], wp[:],
                                 start=True, stop=True)
                nc.scalar.copy(out=S[b][:], in_=Ab[b][:, 128:256])

            SM_CHAIN = (sm_a, sm_b, sm_c, sm_d, sm_e)

            sm0, sm1 = {}, {}
            pass_a(0)
            stats_and_cc(0)
            hooks0 = {3 + i: (lambda f=f: f(0, sm0))
                      for i, f in enumerate(SM_CHAIN)}
            pass_a(1, hooks=hooks0)
            stats_and_cc(1)

            pA_cm.__exit__(None, None, None)

            # -------- fold + pass B: y = (attn.proj)-weighted conv -------
            with tc.tile_pool(name="psout", bufs=1, space="PSUM") as psout:
                def w2_build(b):
                    """W2[i,tap,o] = sum_c Wv[i,tap,c] S[c,o] — 9 small
                    matmuls; taps packed 4+4+1 into two rotating banks."""
                    for grp, taps in enumerate(((0, 1, 2, 3), (4, 5, 6, 7),
                                                (8,))):
                        pw2 = psout.tile([128, 512], F32, tag="pw2",
                                         name="pw2", bufs=2)
                        for j, t in enumerate(taps):
                            nc.tensor.matmul(pw2[:, 128 * j:128 * (j + 1)],
                                             wvt[:, t, :], S[b][:],
                                             start=True, stop=True)
                        dst = W2[b][:, taps[0]:taps[-1] + 1, :]
                        width = 128 * len(taps)
                        if grp == 0:
                            nc.scalar.copy(out=dst, in_=pw2[:, 0:width])
                        else:
                            nc.vector.tensor_copy(out=dst,
                                                  in_=pw2[:, 0:width])

                y3 = y_out[:].rearrange("b p w h -> b p (w h)")

                def conv_out_group(b, g, last_batch):
                    """tap-major folded conv for tiles 2g, 2g+1: one weight
                    load per two 512-row matmuls."""
                    pv = [psout.tile([128, 512], F32, tag="pv", name=f"pv{s}",
                                     bufs=4) for s in range(2)]
                    for t in range(9):
                        k0, k1 = TAPS[t]
                        for s in range(2):
                            nc.tensor.matmul(
                                pv[s][:], W2[b][:, t, :],
                                X[b][:, 4 * g + 2 * s + k0:
                                     4 * g + 2 * s + k0 + 2, k1:k1 + 256],
                                start=(t == 0), stop=(t == 8))
                    ysb = scr.tile([128, 2, 512], BF16, tag="ysb",
                                   name="ysb", bufs=3)
                    for s in range(2):
                        nc.scalar.copy(out=ysb[:, s, 0:256],
                                       in_=pv[s][:, 0:256])
                        nc.vector.tensor_copy(out=ysb[:, s, 256:512],
                                              in_=pv[s][:, 256:512])
                    if last_batch and g == 7:
                        for s in range(2):
                            nc.sync.dma_start(
                                out=y3[b, :, 1024 * g + 512 * s:
                                       1024 * g + 512 * (s + 1)],
                                in_=ysb[:, s, :])
                    else:
                        nc.sync.dma_start(
                            out=y3[b, :, 1024 * g:1024 * (g + 1)],
                            in_=ysb[:])

                sm_f(0, sm0)
                w2_build(0)
                hooksB = {1 + i: (lambda f=f: f(1, sm1))
                          for i, f in enumerate(SM_CHAIN)}
                for g in range(8):
                    conv_out_group(0, g, False)
                    if g in hooksB:
                        hooksB[g]()
                    if g == 6:
                        sm_f(1, sm1)
                w2_build(1)
                for g in range(8):
                    conv_out_group(1, g, True)

            if loop_cm is not None:
                loop_cm.__exit__(None, None, None)

    nc.compile()
    return nc


def _get_nc(reps=0, hwloop=True):
    key = (reps, hwloop)
    if key not in _CACHE:
        _CACHE[key] = _build(reps, hwloop)
    return _CACHE[key]


def _prep_inputs(x, qkv_wr, qkv_wi, dw_wr, dw_wi, proj_wr, proj_wi):
    import ml_dtypes
    bf16 = ml_dtypes.bfloat16
    f8 = mybir.dt.np(FP8)

    cw = np.complex128
    Q = (qkv_wr[:, :, 0, 0].astype(np.float64)
         + 1j * qkv_wi[:, :, 0, 0].astype(np.float64))
    D = (dw_wr[:, 0].astype(np.float64)
         + 1j * dw_wi[:, 0].astype(np.float64)).reshape(3 * C, 9).astype(cw)
    w_full = np.zeros((128, 9, 3, 128), np.float32)
    for t in range(9):
        F = D[:, t:t + 1] * Q            # [192, 64] complex
        for chunk in range(3):
            Fc = F[64 * chunk:64 * (chunk + 1)]   # [64 out, 64 in]
            Re, Im = Fc.real.T, Fc.imag.T          # [in, out]
            w_full[:, t, chunk, :] = np.block([[Re, Im], [-Im, Re]])
    # chunk order in w_full: 0=q, 1=k, 2=v.  v weights ship TRANSPOSED
    # ([cin, tap, cout] -> [c, tap, i]) as the lhs of the on-device fold.
    w_vt = np.ascontiguousarray(
        w_full[:, :, 2, :].transpose(2, 1, 0)).astype(bf16)
    w_qk9 = np.ascontiguousarray(
        w_full[:, :, 0:2, :].transpose(0, 2, 1, 3)) * WSCALE   # [128,2,9,128]
    w_qk = np.zeros((128, 2, 10, 128), np.float32)
    w_qk[:, :, 0:9, :] = w_qk9
    w_qk = w_qk.astype(f8)
    P = (proj_wr[:, :, 0, 0].astype(np.float64)
         + 1j * proj_wi[:, :, 0, 0].astype(np.float64))
    Re, Im = P.real.T, P.imag.T
    w_proj = np.block([[Re, Im], [-Im, Re]]).astype(bf16)

    xpad = np.pad(np.asarray(x, np.float32),
                  ((0, 0), (0, 0), (1, 1), (0, 0), (0, 0)))
    in_maps = []
    for core in range(NCORES):
        xs = xpad[:, :, WL * core:WL * core + WLH, :, :]
        xc = np.zeros((B, 128, WLH, HP), np.float32)
        xc[:, :C, :, 1:H + 1] = xs[..., 0]
        xc[:, C:, :, 1:H + 1] = xs[..., 1]
        in_maps.append({"x_bf": xc.astype(bf16), "x_f8": xc.astype(f8),
                        "w_vt": w_vt, "w_qk": w_qk, "w_proj": w_proj})
    return in_maps


def _assemble(results):
    out = np.empty((B, C, W, H, 2), np.float32)
    for core in range(NCORES):
        o = results[core]["y_out"].astype(np.float32).reshape(B, 2, C, WL, H)
        out[:, :, WL * core:WL * (core + 1), :, :] = o.transpose(0, 2, 3, 4, 1)
    return out


def kernel(x, qkv_wr, qkv_wi, dw_wr, dw_wi, proj_wr, proj_wi):
    nc = _get_nc()
    in_maps = _prep_inputs(x, qkv_wr, qkv_wi, dw_wr, dw_wi,
                           proj_wr, proj_wi)
    res = run_bass_kernel_spmd(nc, in_maps, list(range(NCORES)))
    return _assemble(res.results)


# revision 4
# speedup vs baseline: 1.0098x; 1.0098x over previous
"""Trainium2 Bass kernel for complex-valued channel attention (XCA-style) — v4.

Two-pass restructure of v3, designed around the fact that walrus compiles
with --enable-ldw-opt=false: EVERY matmul reloads its stationary weights,
so the kernel keeps each weight load shorter than the compute issued per
load (2+ matmuls per load), instead of the baseline's per-row qk matmuls
(one 256-col DR load per 129-cycle matmul).

  Pass A (per batch): q,k conv in fp8 DoubleRow over FLAT 512-wide chunks
    of the padded plane (output positions at row-crossings are garbage and
    simply never read; token blocks of 128 stay contiguous).  Pair-major
    over 2-chunk groups: one DR weight load feeds two 256-cycle matmuls.
    Transposes + QK^T/QQ^T accumulation pipeline behind the conv exactly
    as before ([kT|qT]-interleaved fp8 blocks, DR token-pair matmuls).

  Fold: S = attn @ w_proj (softmax stages on DVE/ACT, fast-rsqrt norms on
    DVE — no ACT table switches), then W2[i,tap,o] = sum_c Wv[i,tap,c]
    S[c,o] built on-device with 9 small matmuls.

  Pass B (per batch): y = W2-conv(x) directly — the 9-tap bf16 v-conv with
    attn+proj pre-folded into its weights.  Tap-major over 2-tile groups
    (one 128-col load per two 512-row matmuls), output evicted straight to
    the y staging buffer.  The separate attn@v tail phase, the V SBUF
    buffer, and its PSUM evictions are gone.
"""

import sys

sys.path.insert(0, '/opt/trn_rl_repo')

import numpy as np

import concourse.bass as bass  # noqa: F401  (registers bass types)
import concourse.tile as tile
from concourse import bacc, mybir
from concourse.ap import AP
from concourse.bass_utils import run_bass_kernel_spmd
from concourse.masks import make_identity

F32 = mybir.dt.float32
BF16 = mybir.dt.bfloat16
FP8 = mybir.dt.float8e4
I32 = mybir.dt.int32

B, C, W, H = 2, 64, 256, 256
NCORES = 8
WL = W // NCORES          # 32 local w rows per core
HP = H + 2                # 258: h with zero pad columns
WLH = WL + 2              # 34: local w rows + halo
NWT = WL // 2             # 16 tiles of 512 tokens (2 w-rows x 256 h)
FLAT = WL * HP            # 8256 flat conv output positions per batch
EPS = 1e-12
WSCALE = 64.0             # q,k fp8 weights are scaled by 2^6

# tap order: it = 3*k0 + k1;  DR pairs (0,1) (2,3) (4,5) (6,7) (8,zero)
TAPS = [(k0, k1) for k0 in range(3) for k1 in range(3)]

# pass-A chunk groups: 8 pairs of 512-flat chunks (one DR weight load per
# two matmuls), then the 64-wide tail chunk solo (reuses the pair tags)
GROUPS = [((2 * g, 512), (2 * g + 1, 512)) for g in range(8)]
GROUPS.append(((16, 64),))

_CACHE = {}


def _build(reps=0, hwloop=True):
    """Emit + compile the 8-core SPMD program. reps>0 wraps the compute in a
    hardware loop (used only for timing; collectives become local copies)."""
    nc = bacc.Bacc(None, target_bir_lowering=False, debug=False,
                   num_devices=NCORES)
    x_bf = nc.declare_dram_parameter("x_bf", [B, 128, WLH, HP], BF16,
                                     isOutput=False)
    x_f8 = nc.declare_dram_parameter("x_f8", [B, 128, WLH, HP], FP8,
                                     isOutput=False)
    w_qk = nc.declare_dram_parameter("w_qk", [128, 2, 10, 128], FP8,
                                     isOutput=False)
    w_vt = nc.declare_dram_parameter("w_vt", [128, 9, 128], BF16,
                                     isOutput=False)
    w_proj = nc.declare_dram_parameter("w_proj", [128, 128], BF16,
                                       isOutput=False)
    y_out = nc.declare_dram_parameter("y_out", [B, 128, WL, H], BF16,
                                      isOutput=True)

    with tile.TileContext(nc) as tc:
        with (
            tc.tile_pool(name="const", bufs=1) as const,
            tc.tile_pool(name="xp", bufs=1) as xp,
            tc.tile_pool(name="qs", bufs=1) as qsp,
            tc.tile_pool(name="qkt", bufs=3) as qkt,
            tc.tile_pool(name="w2p", bufs=1) as w2p,
            tc.tile_pool(name="scr", bufs=2) as scr,
            tc.tile_pool(name="stat", bufs=1) as stat,
            tc.tile_pool(name="dram", bufs=1, space="DRAM") as dram,
            tc.tile_pool(name="psacc", bufs=1, space="PSUM") as psacc,
        ):
            wqk = const.tile([128, 2, 10, 128], FP8)
            wvt = const.tile([128, 9, 128], BF16)
            wp = const.tile([128, 128], BF16)
            identf = const.tile([128, 128], F32)
            make_identity(nc, identf[:])
            identb = const.tile([128, 128], BF16)
            nc.vector.tensor_copy(out=identb[:], in_=identf[:])

            # pass A reads the flat fp8 copy (with a 2-element guard for
            # pad-position reads past the data end); pass B the bf16 one.
            # fp8 chunks stream first: they gate time-to-first-matmul.
            X = [xp.tile([128, WLH, HP], BF16, tag=f"x{b}", name=f"X{b}")
                 for b in range(B)]
            X8 = [xp.tile([128, WLH * HP + 2], FP8, tag=f"x8{b}",
                          name=f"X8{b}")
                  for b in range(B)]
            CHUNKS = ((0, 4), (4, 8), (8, 14), (14, 21), (21, 28),
                      (28, WLH))

            def x8_dma(b, lo, hi):
                x8v = X8[b][:, lo * HP:hi * HP].rearrange(
                    "p (w h) -> p w h", h=HP)
                nc.sync.dma_start(out=x8v, in_=x_f8[b, :, lo:hi, :])

            # conv group 0 reads w-rows 0..5, so the head of the queue is
            # exactly two DMAs: one 8-row x8 chunk + the conv weights
            x8_dma(0, 0, 8)
            nc.sync.dma_start(out=wqk[:], in_=w_qk[:])
            for lo, hi in CHUNKS[2:]:
                x8_dma(0, lo, hi)
            for lo, hi in CHUNKS:
                x8_dma(1, lo, hi)
            for b in range(B):
                nc.vector.memset(X8[b][:, WLH * HP:WLH * HP + 2], 0)
            nc.sync.dma_start(out=wvt[:], in_=w_vt[:])
            nc.sync.dma_start(out=wp[:], in_=w_proj[:])
            for b in range(B):
                for lo, hi in CHUNKS:
                    nc.sync.dma_start(out=X[b][:, lo:hi, :],
                                      in_=x_bf[b, :, lo:hi, :])

            # flat fp8 conv outputs (token rows 258 apart, 2 garbage cols
            # per row); one buffer per stream, reused across batches
            q_s8 = qsp.tile([128, FLAT], BF16, name="q_s8")
            k_s8 = qsp.tile([128, FLAT], BF16, name="k_s8")
            # per-batch QK accumulator banks (double as softmax scratch)
            Ab = [psacc.tile([128, 384], F32, tag=f"acc{b}", name=f"A{b}")
                  for b in range(B)]
            S = [stat.tile([128, 128], BF16, tag=f"S{b}", name=f"S{b}")
                 for b in range(B)]
            W2 = [stat.tile([128, 9, 128], BF16, tag=f"W2{b}",
                            name=f"W2{b}") for b in range(B)]
            stats_s = [stat.tile([128, 130], F32, tag=f"st{b}", name=f"st{b}")
                       for b in range(B)]
            rstats = [stat.tile([128, 130], F32, tag=f"rst{b}", name=f"rst{b}")
                      for b in range(B)]
            cc_in = [dram.tile([128, 130], F32, tag=f"ci{b}", name=f"ci{b}")
                     for b in range(B)]
            cc_out = [dram.tile([128, 130], F32, tag=f"co{b}", name=f"co{b}")
                      for b in range(B)]

            def dr_rhs(xt, flat_off, width, pair):
                """[128, 2(tap delta), width] overlapping AP over the flat
                padded plane: DR pair input windows for output flat
                positions [flat_off, flat_off+width)."""
                k0a, k1a = TAPS[pair[0]]
                full = xt[:]
                pstride = full.ap[0][0]
                off = full.offset + flat_off + k0a * HP + k1a
                if pair[1] < 9:
                    k0b, k1b = TAPS[pair[1]]
                    d = (k0b - k0a) * HP + (k1b - k1a)
                else:
                    d = 0
                return AP(full.tensor, off, [[pstride, 128], [d, 2],
                                             [1, width]])

            loop_cm = (tc.For_i(0, reps, 1,
                                hint_engines=(mybir.EngineType.PE,
                                              mybir.EngineType.Activation,
                                              mybir.EngineType.DVE))
                       if reps and hwloop else None)
            if loop_cm is not None:
                loop_cm.__enter__()

            # ---------------- pass A: q,k conv + QK^T ----------------
            pA_cm = tc.tile_pool(name="pA", bufs=1, space="PSUM")
            pA = pA_cm.__enter__()

            qkt_hist = {}

            def conv_group(b, g):
                """fp8 DR conv for one chunk group, pair-major: one weight
                load per 2-3 chunk matmuls."""
                chunks = GROUPS[g]
                pq = [pA.tile([128, 512], F32, tag=f"pq{i}", name=f"pq{i}",
                              bufs=1) for i, _ in enumerate(chunks)]
                pk = [pA.tile([128, 512], F32, tag=f"pk{i}", name=f"pk{i}",
                              bufs=1) for i, _ in enumerate(chunks)]
                for chunk, pt in ((0, pq), (1, pk)):
                    for p in range(5):
                        for i, (c, width) in enumerate(chunks):
                            nc.tensor.matmul(
                                pt[i][:, 0:width],
                                wqk[:, chunk, 2 * p:2 * p + 2, :],
                                dr_rhs(X8[b], 512 * c, width,
                                       (2 * p, 2 * p + 1)),
                                start=(p == 0), stop=(p == 4),
                                perf_mode=mybir.MatmulPerfMode.DoubleRow)
                for i, (c, width) in enumerate(chunks):
                    nc.vector.tensor_scalar_mul(
                        out=q_s8[:, 512 * c:512 * c + width],
                        in0=pq[i][:, 0:width], scalar1=1.0 / WSCALE)
                    nc.scalar.activation(
                        out=k_s8[:, 512 * c:512 * c + width],
                        in_=pk[i][:, 0:width],
                        func=mybir.ActivationFunctionType.Copy,
                        scale=1.0 / WSCALE)

            def tr_stage(b, wt):
                """8 transposes of tile wt's token blocks into one
                [kT|qT]-interleaved fp8 PSUM bank, then evict."""
                ptqk = pA.tile([128, 1024], BF16, tag="ptqk", name="ptqk",
                               bufs=2)
                base = 516 * wt
                offs = (base, base + 128, base + 258, base + 386)
                for j, o in enumerate(offs):
                    nc.tensor.transpose(ptqk[:, 256 * j:256 * j + 128],
                                        k_s8[:, o:o + 128], identb[:])
                    nc.tensor.transpose(ptqk[:, 256 * j + 128:256 * (j + 1)],
                                        q_s8[:, o:o + 128], identb[:])
                QKT = qkt.tile([128, 1024], FP8, tag="QKT", name="QKT")
                nc.scalar.copy(out=QKT[:, 0:512], in_=ptqk[:, 0:512])
                nc.vector.tensor_copy(out=QKT[:, 512:1024],
                                      in_=ptqk[:, 512:1024])
                qkt_hist[wt] = QKT

            def qk_stage(b, wt):
                QKT = qkt_hist.pop(wt)
                blk = QKT[:].rearrange("p (j c) -> p j c", j=4)
                # [QK|QQ] plus KK (its diagonal gives the k norms, so no
                # ACT Square pass loads the eviction engines).  All writes
                # are ONE per-bank accumulation group: start only on the
                # very first matmul, stop only on the very last.
                for j in (0, 2):
                    nc.tensor.matmul(
                        Ab[b][:, 0:256],
                        blk[:, j:j + 2, 128:256], blk[:, j:j + 2, :],
                        start=(wt == 0 and j == 0), stop=False,
                        perf_mode=mybir.MatmulPerfMode.DoubleRow)
                    nc.tensor.matmul(
                        Ab[b][:, 256:384],
                        blk[:, j:j + 2, 0:128], blk[:, j:j + 2, 0:128],
                        start=False,
                        stop=(wt == NWT - 1 and j == 2),
                        perf_mode=mybir.MatmulPerfMode.DoubleRow)

            def pass_a(b, hooks=None):
                # tiles become transposable once the chunks covering their
                # 516-wide flat span are evicted (2-ish per group)
                done_tr = 0
                done_qk = 0
                for g in range(len(GROUPS) + 1):
                    if g < len(GROUPS):
                        conv_group(b, g)
                    hi_flat = 1024 * (g + 1) if g < len(GROUPS) else 99999
                    while done_tr < NWT and 516 * (done_tr + 1) <= hi_flat:
                        tr_stage(b, done_tr)
                        done_tr += 1
                        # qk matmuls lag one tile behind their transpose
                        if done_qk < done_tr - 1:
                            qk_stage(b, done_qk)
                            done_qk += 1
                    if hooks and g in hooks:
                        hooks[g]()
                while done_qk < NWT:
                    qk_stage(b, done_qk)
                    done_qk += 1

            def stats_and_cc(b):
                # local [QK | diag(QQ) | sum k^2] -> AllReduce across cores
                nc.scalar.copy(out=stats_s[b][:, 0:128], in_=Ab[b][:, 0:128])
                dscr = scr.tile([128, 128], F32, tag="dscr", name="dscr")
                nc.vector.tensor_tensor(out=dscr[:], in0=Ab[b][:, 128:256],
                                        in1=identf[:],
                                        op=mybir.AluOpType.mult)
                nc.vector.reduce_sum(out=stats_s[b][:, 128:129], in_=dscr[:],
                                     axis=mybir.AxisListType.X)
                dscr2 = scr.tile([128, 128], F32, tag="dscr2", name="dscr2")
                nc.vector.tensor_tensor(out=dscr2[:], in0=Ab[b][:, 256:384],
                                        in1=identf[:],
                                        op=mybir.AluOpType.mult)
                nc.vector.reduce_sum(out=stats_s[b][:, 129:130],
                                     in_=dscr2[:],
                                     axis=mybir.AxisListType.X)
                if reps:
                    nc.vector.tensor_copy(out=rstats[b][:], in_=stats_s[b][:])
                else:
                    nc.sync.dma_start(out=cc_in[b][:], in_=stats_s[b][:])
                    nc.gpsimd.collective_compute(
                        "AllReduce", mybir.AluOpType.add,
                        replica_groups=[list(range(NCORES))],
                        ins=[cc_in[b].opt()], outs=[cc_out[b].opt()],
                    )
                    nc.sync.dma_start(out=rstats[b][:], in_=cc_out[b][:])

            def w2_build(b, pool, tags):
                """W2[i,tap,o] = sum_c Wv[i,tap,c] S[c,o] — 9 small matmuls;
                taps packed 4+4+1 into rotating banks of `pool`."""
                for grp, taps in enumerate(((0, 1, 2, 3), (4, 5, 6, 7),
                                            (8,))):
                    pw2 = pool.tile([128, 512], F32, tag=tags[grp % len(tags)],
                                    name="pw2", bufs=1 if len(tags) > 1 else 2)
                    for j, t in enumerate(taps):
                        nc.tensor.matmul(pw2[:, 128 * j:128 * (j + 1)],
                                         wvt[:, t, :], S[b][:],
                                         start=True, stop=True)
                    dst = W2[b][:, taps[0]:taps[-1] + 1, :]
                    width = 128 * len(taps)
                    if grp == 0:
                        nc.scalar.copy(out=dst, in_=pw2[:, 0:width])
                    else:
                        nc.vector.tensor_copy(out=dst, in_=pw2[:, 0:width])

            # ---- softmax stages (split fine so PE ops issue one hook
            # after the V-engine ops they depend on) ----
            def sm_a(b, st):
                # V-engine only: reciprocal norms
                nq = scr.tile([128, 2], F32, tag="nq", name="nq")
                nc.scalar.activation(
                    out=nq[:], in_=rstats[b][:, 128:130],
                    func=mybir.ActivationFunctionType.Sqrt)
                nqm = scr.tile([128, 2], F32, tag="nqm", name="nqm")
                nc.vector.tensor_scalar_max(out=nqm[:], in0=nq[:],
                                            scalar1=EPS)
                rqk = scr.tile([128, 2], F32, tag="rqk", name="rqk",
                               bufs=2)
                nc.vector.reciprocal(out=rqk[:], in_=nqm[:])
                st['rqk'] = rqk

            def sm_b(b, st):
                nc.tensor.transpose(Ab[b][:, 0:128], rstats[b][:, 0:128],
                                    identf[:])
                s1 = scr.tile([128, 128], F32, tag="s1", name="s1", bufs=2)
                nc.scalar.copy(out=s1[:], in_=Ab[b][:, 0:128])
                st['s1'] = s1

            def sm_c(b, st):
                s1b = scr.tile([128, 128], F32, tag="s1b", name="s1b",
                               bufs=2)
                nc.vector.tensor_scalar_mul(out=s1b[:], in0=st['s1'][:],
                                            scalar1=st['rqk'][:, 1:2])
                st['s1b'] = s1b

            def sm_d(b, st):
                nc.tensor.transpose(Ab[b][:, 128:256], st['s1b'][:],
                                    identf[:])
                lg = scr.tile([128, 128], F32, tag="lg", name="lg", bufs=2)
                nc.scalar.copy(out=lg[:], in_=Ab[b][:, 128:256])
                st['lg'] = lg

            def sm_e(b, st):
                lg2 = scr.tile([128, 128], F32, tag="lg2", name="lg2")
                nc.vector.tensor_scalar_mul(out=lg2[:], in0=st['lg'][:],
                                            scalar1=st['rqk'][:, 0:1])
                nc.vector.memset(lg2[0:64, 64:128], -1e30)
                nc.vector.memset(lg2[64:128, 0:64], -1e30)
                mx = scr.tile([128, 1], F32, tag="mx", name="mx")
                nc.vector.reduce_max(out=mx[:], in_=lg2[:],
                                     axis=mybir.AxisListType.X)
                sh = scr.tile([128, 128], F32, tag="sh", name="sh")
                nc.vector.tensor_scalar(out=sh[:], in0=lg2[:], scalar1=mx[:],
                                        scalar2=None,
                                        op0=mybir.AluOpType.subtract)
                ex = scr.tile([128, 128], F32, tag="ex", name="ex")
                esum = scr.tile([128, 1], F32, tag="esum", name="esum")
                nc.scalar.activation(out=ex[:], in_=sh[:],
                                     func=mybir.ActivationFunctionType.Exp,
                                     accum_out=esum[:])
                rs = scr.tile([128, 1], F32, tag="rs", name="rs")
                nc.vector.reciprocal(out=rs[:], in_=esum[:])
                pr = scr.tile([128, 128], BF16, tag="pr", name="pr",
                              bufs=2)
                nc.vector.tensor_scalar_mul(out=pr[:], in0=ex[:],
                                            scalar1=rs[:])
                st['pr'] = pr

            def sm_f(b, st):
                nc.tensor.matmul(Ab[b][:, 128:256], st['pr'][:], wp[:],
                                 start=True, stop=True)
                nc.scalar.copy(out=S[b][:], in_=Ab[b][:, 128:256])

            SM_CHAIN = (sm_a, sm_b, sm_c, sm_d, sm_e)

            sm0, sm1 = {}, {}
            pass_a(0)
            stats_and_cc(0)
            hooks0 = {2 + i: (lambda f=f: f(0, sm0))
                      for i, f in enumerate(SM_CHAIN)}
            hooks0[7] = lambda: sm_f(0, sm0)
            hooks0[8] = lambda: w2_build(0, pA, ("pq0", "pq1"))
            pass_a(1, hooks=hooks0)
            stats_and_cc(1)

            pA_cm.__exit__(None, None, None)

            # -------- fold + pass B: y = (attn.proj)-weighted conv -------
            with tc.tile_pool(name="psout", bufs=1, space="PSUM") as psout:
                y3 = y_out[:].rearrange("b p w h -> b p (w h)")

                def conv_out_group(b, g, last_batch):
                    """tap-major folded conv for tiles 2g, 2g+1: one weight
                    load per two 512-row matmuls."""
                    pv = [psout.tile([128, 512], F32, tag="pv", name=f"pv{s}",
                                     bufs=4) for s in range(2)]
                    for t in range(9):
                        k0, k1 = TAPS[t]
                        for s in range(2):
                            nc.tensor.matmul(
                                pv[s][:], W2[b][:, t, :],
                                X[b][:, 4 * g + 2 * s + k0:
                                     4 * g + 2 * s + k0 + 2, k1:k1 + 256],
                                start=(t == 0), stop=(t == 8))
                    ysb = scr.tile([128, 2, 512], BF16, tag="ysb",
                                   name="ysb", bufs=3)
                    for s in range(2):
                        nc.scalar.copy(out=ysb[:, s, 0:256],
                                       in_=pv[s][:, 0:256])
                        nc.vector.tensor_copy(out=ysb[:, s, 256:512],
                                              in_=pv[s][:, 256:512])
                    if last_batch and g == 7:
                        for s in range(2):
                            nc.sync.dma_start(
                                out=y3[b, :, 1024 * g + 512 * s:
                                       1024 * g + 512 * (s + 1)],
                                in_=ysb[:, s, :])
                    else:
                        nc.sync.dma_start(
                            out=y3[b, :, 1024 * g:1024 * (g + 1)],
                            in_=ysb[:])

                hooksB = {i: (lambda f=f: f(1, sm1))
                          for i, f in enumerate(SM_CHAIN)}
                for g in range(8):
                    conv_out_group(0, g, False)
                    if g in hooksB:
                        hooksB[g]()
                    if g == 5:
                        sm_f(1, sm1)
                    if g == 6:
                        w2_build(1, psout, ("pw2",))
                for g in range(8):
                    conv_out_group(1, g, True)

            if loop_cm is not None:
                loop_cm.__exit__(None, None, None)

    nc.compile()
    return nc


def _get_nc(reps=0, hwloop=True):
    key = (reps, hwloop)
    if key not in _CACHE:
        _CACHE[key] = _build(reps, hwloop)
    return _CACHE[key]


def _prep_inputs(x, qkv_wr, qkv_wi, dw_wr, dw_wi, proj_wr, proj_wi):
    import ml_dtypes
    bf16 = ml_dtypes.bfloat16
    f8 = mybir.dt.np(FP8)

    cw = np.complex128
    Q = (qkv_wr[:, :, 0, 0].astype(np.float64)
         + 1j * qkv_wi[:, :, 0, 0].astype(np.float64))
    D = (dw_wr[:, 0].astype(np.float64)
         + 1j * dw_wi[:, 0].astype(np.float64)).reshape(3 * C, 9).astype(cw)
    w_full = np.zeros((128, 9, 3, 128), np.float32)
    for t in range(9):
        F = D[:, t:t + 1] * Q            # [192, 64] complex
        for chunk in range(3):
            Fc = F[64 * chunk:64 * (chunk + 1)]   # [64 out, 64 in]
            Re, Im = Fc.real.T, Fc.imag.T          # [in, out]
            w_full[:, t, chunk, :] = np.block([[Re, Im], [-Im, Re]])
    # chunk order in w_full: 0=q, 1=k, 2=v.  v weights ship TRANSPOSED
    # ([cin, tap, cout] -> [c, tap, i]) as the lhs of the on-device fold.
    w_vt = np.ascontiguousarray(
        w_full[:, :, 2, :].transpose(2, 1, 0)).astype(bf16)
    w_qk9 = np.ascontiguousarray(
        w_full[:, :, 0:2, :].transpose(0, 2, 1, 3)) * WSCALE   # [128,2,9,128]
    w_qk = np.zeros((128, 2, 10, 128), np.float32)
    w_qk[:, :, 0:9, :] = w_qk9
    w_qk = w_qk.astype(f8)
    P = (proj_wr[:, :, 0, 0].astype(np.float64)
         + 1j * proj_wi[:, :, 0, 0].astype(np.float64))
    Re, Im = P.real.T, P.imag.T
    w_proj = np.block([[Re, Im], [-Im, Re]]).astype(bf16)

    xpad = np.pad(np.asarray(x, np.float32),
                  ((0, 0), (0, 0), (1, 1), (0, 0), (0, 0)))
    in_maps = []
    for core in range(NCORES):
        xs = xpad[:, :, WL * core:WL * core + WLH, :, :]
        xc = np.zeros((B, 128, WLH, HP), np.float32)
        xc[:, :C, :, 1:H + 1] = xs[..., 0]
        xc[:, C:, :, 1:H + 1] = xs[..., 1]
        in_maps.append({"x_bf": xc.astype(bf16), "x_f8": xc.astype(f8),
                        "w_vt": w_vt, "w_qk": w_qk, "w_proj": w_proj})
    return in_maps


def _assemble(results):
    out = np.empty((B, C, W, H, 2), np.float32)
    for core in range(NCORES):
        o = results[core]["y_out"].astype(np.float32).reshape(B, 2, C, WL, H)
        out[:, :, WL * core:WL * (core + 1), :, :] = o.transpose(0, 2, 3, 4, 1)
    return out


def kernel(x, qkv_wr, qkv_wi, dw_wr, dw_wi, proj_wr, proj_wi):
    nc = _get_nc()
    in_maps = _prep_inputs(x, qkv_wr, qkv_wi, dw_wr, dw_wi,
                           proj_wr, proj_wi)
    res = run_bass_kernel_spmd(nc, in_maps, list(range(NCORES)))
    return _assemble(res.results)
